# revision 13
# baseline (speedup 1.0000x reference)
"""MoE routing gate kernel for Trainium2 (8 NeuronCores, data-parallel).

Computes, for x[32768, 2048], weight[64, 2048], bias[64]:
    logits = x @ weight.T
    probs  = softmax(logits, axis=-1)
    idx    = top_k(probs + bias, 6).indices
    w      = take_along_axis(probs, idx)
returning (w float32 [32768, 6], idx int32 [32768, 6]).

Sharding: tokens split 4096/core across 8 cores; weight/bias replicated.

Per-core pipeline (memory-bound). HBM traffic is the lever: instead of
shipping x at 4 B/elem (fp32 or bf16 hi+lo), the host re-encodes x as
fp16 hi (2 B) + a scaled fp8e4m3 residual (1 B) -- 25.2 MB/shard, a
~70.5 us DMA floor at 360 GB/s vs ~94 us for 4 B/elem.  The logits are
reconstructed on-device to ~4e-6 rms (max ~4e-5) absolute error:

  - Stationary for the fp16 pass packs BOTH weight precision levels in
    the PE array's 128 columns: cols 0-63 = fp16(w), cols 64-127 =
    (w - fp16(w)) * 2^22 in fp16 (the scale keeps the tiny residual out
    of fp16's denormal range).  One pass over x_hi yields w_hi @ x_hi in
    PSUM partitions 0-63 and 2^22 * w_lo @ x_hi in partitions 64-127.
  - The x residual pass is fp8e4m3 x fp8e4m3: lo8 = fp8(r * 2^16) with
    w8 = fp8(w * 2^6); the product scale (2^22) matches the lo partition
    group, so it accumulates straight into partitions 64-127.
  - One PE transpose per 128-token tile (identity = eye(128)) moves both
    groups back to token-major; a single DVE op fuses the recombine
    logit = hi + 2^-22 * lo.
  - Softmax without max-subtraction (|logits| < ~7): ACT exp emits the
    row sum via accum_out.  Selection key q = exp + sum*bias ranks
    identically to probs + bias.
  - DVE Max8/MaxIndex8 give top-8 values+indices; the top-6 unbiased
    weights come from 6 fused scalar_tensor_tensor gathers
    ((iota == idx_k) * exp with accum_out), split across DVE and the
    otherwise-idle Pool engine, then one scale by 1/sum.

x_hi DMAs ride the SP queue, x_lo the ACT queue (an issuing engine is
occupied for its transfer, so the two streams interleave at the DMA
device and the per-DMA fixed costs hide under each other).
"""

import numpy as np
import ml_dtypes

import concourse.bacc as bacc
import concourse.bass as bass
import concourse.mybir as mybir
import concourse.tile as tile
from concourse.bass_utils import run_bass_kernel_spmd

F16 = mybir.dt.float16
F8 = mybir.dt.float8e4
BF16 = mybir.dt.bfloat16
F32 = mybir.dt.float32
I32 = mybir.dt.int32
U16 = mybir.dt.uint16
OP = mybir.AluOpType
EXP = mybir.ActivationFunctionType.Exp

TOKENS, DIM, E, TOPK, NCORES = 32768, 2048, 64, 6, 8
KC = DIM // 128   # contraction chunks of 128
SGT = 512         # tokens per super-group (= matmul N, one PSUM bank)
KQH = 8           # fp16 k-chunks per DMA (2 DMAs per sg)
LO_SCALE = float(2.0 ** -22)


def build_nc(tpc, sgt=SGT):
    """Build the per-core Bass program for a tpc-token shard."""
    nsg = tpc // sgt
    nj = sgt // 128       # 128-token tiles per super-group
    cols = nj * TOPK      # staging cols per sg

    nc = bacc.Bacc("TRN2", target_bir_lowering=False, debug=False)

    xh = nc.dram_tensor("xh", [nsg, KC // KQH, 128, KQH, sgt], F16, kind="ExternalInput")
    xl = nc.dram_tensor("xl", [nsg, 128, KC, sgt], F8, kind="ExternalInput")
    wa = nc.dram_tensor("wa", [128, KC, 128], F16, kind="ExternalInput")
    w8 = nc.dram_tensor("w8", [128, KC, E], F8, kind="ExternalInput")
    # eye(64) fp32 for the hi transpose; eye(64)*2^-22 in bf16 (exact: power
    # of two) for the lo-group recombine matmul.  The recombine must NOT be
    # an fp32 matmul: a regular fp32 matmul following FWL-loaded f16/f8
    # matmuls hangs the PE (the LastMatmultFP32HI erratum); bf16 is safe.
    ident = nc.dram_tensor("ident", [64, 64], F32, kind="ExternalInput")
    identlo = nc.dram_tensor("identlo", [64, 64], BF16, kind="ExternalInput")
    bias_b = nc.dram_tensor("bias_b", [128, E], F32, kind="ExternalInput")
    iota64 = nc.dram_tensor("iota64", [128, E], F32, kind="ExternalInput")
    w_out = nc.dram_tensor("w_out", [nsg, 128, cols], F32, kind="ExternalOutput")
    i_out = nc.dram_tensor("i_out", [nsg, 128, cols], I32, kind="ExternalOutput")

    with tile.TileContext(nc) as tc:
        with (
            tc.tile_pool(name="consts", bufs=1) as cpool,
            tc.tile_pool(name="xhbuf", bufs=5) as xhp,
            tc.tile_pool(name="xlbuf", bufs=3) as xlp,
            tc.tile_pool(name="lt", bufs=2) as ltp,
            tc.tile_pool(name="small", bufs=8) as smp,
            tc.tile_pool(name="work", bufs=4) as wkp,
            tc.tile_pool(name="stage", bufs=3) as stp,
            tc.tile_pool(name="acc", bufs=2, space="PSUM") as accp,
            tc.tile_pool(name="tr", bufs=4, space="PSUM") as trp,
        ):
            cwa = cpool.tile([128, KC, 128], F16)
            nc.gpsimd.dma_start(cwa, wa[:])
            cw8 = cpool.tile([128, KC, E], F8)
            nc.gpsimd.dma_start(cw8, w8[:])
            cident = cpool.tile([64, 64], F32)
            nc.gpsimd.dma_start(cident, ident[:])
            cidlo = cpool.tile([64, 64], BF16)
            nc.gpsimd.dma_start(cidlo, identlo[:])
            cbias = cpool.tile([128, E], F32)
            nc.gpsimd.dma_start(cbias, bias_b[:])
            ciota = cpool.tile([128, E], F32)
            nc.gpsimd.dma_start(ciota, iota64[:])

            for sg in range(nsg):
                xh0 = xhp.tile([128, KQH, sgt], F16, tag="xh")
                nc.sync.dma_start(xh0, xh[sg, 0])
                xlt = xlp.tile([128, KC, sgt], F8, tag="xl")
                nc.scalar.dma_start(xlt, xl[sg])
                xh1 = xhp.tile([128, KQH, sgt], F16, tag="xh")
                nc.sync.dma_start(xh1, xh[sg, 1])

                acc = accp.tile([128, sgt], F32)
                for k in range(KQH):
                    nc.tensor.matmul(
                        acc, cwa[:, k, :], xh0[:, k, :],
                        start=(k == 0), stop=False,
                    )
                for k in range(KC):
                    nc.tensor.matmul(
                        acc[64:128], cw8[:, k, :], xlt[:, k, :],
                        start=False, stop=False,
                    )
                for k in range(KQH, KC):
                    nc.tensor.matmul(
                        acc, cwa[:, k, :], xh1[:, k - KQH, :],
                        start=False, stop=(k == KC - 1),
                    )

                lth = ltp.tile([64, sgt], F32, tag="lth")
                nc.scalar.copy(lth, acc[0:64])
                ltl = ltp.tile([64, sgt], BF16, tag="ltl")
                nc.scalar.copy(ltl, acc[64:128])

                sw = stp.tile([128, cols], F32, tag="sw")
                si = stp.tile([128, cols], I32, tag="si")
                for j in range(nj):
                    chunk = slice(j * 128, (j + 1) * 128)
                    tps = trp.tile([128, E], F32)
                    nc.tensor.matmul(
                        tps, lth[:, chunk], cident,
                        is_transpose=True, start=True, stop=False,
                    )
                    nc.tensor.matmul(
                        tps, ltl[:, chunk], cidlo,
                        start=False, stop=True,
                    )
                    ex = wkp.tile([128, E], F32, tag="ex", bufs=nj + 2)
                    ssum = smp.tile([128, 1], F32, tag="ssum")
                    nc.scalar.activation(ex, tps, EXP, accum_out=ssum)
                    q = wkp.tile([128, E], F32, tag="q")
                    nc.vector.scalar_tensor_tensor(
                        q, cbias, ssum, ex, OP.mult, OP.add
                    )
                    mx = smp.tile([128, 8], F32, tag="mx")
                    nc.vector.max(mx, q)
                    mi = smp.tile([128, 8], U16, tag="mi")
                    nc.vector.max_index(mi, mx, q)
                    idxf = smp.tile([128, 8], F32, tag="idxf")
                    nc.vector.tensor_copy(idxf, mi)
                    rs = smp.tile([128, 1], F32, tag="rs")
                    nc.vector.reciprocal(rs, ssum)
                    col = j * TOPK
                    nc.vector.tensor_copy(si[:, col:col + TOPK], mi[:, 0:TOPK])
                    scr = wkp.tile([128, TOPK, E], F32, tag="scr")
                    g6 = smp.tile([128, TOPK], F32, tag="g6")
                    for kk in range(TOPK):
                        nc.vector.scalar_tensor_tensor(
                            scr[:, kk], ciota, idxf[:, kk:kk + 1], ex,
                            OP.is_equal, OP.mult,
                            accum_out=g6[:, kk:kk + 1],
                        )
                    nc.vector.tensor_scalar_mul(sw[:, col:col + TOPK], g6, rs)
                nc.gpsimd.dma_start(w_out[sg], sw)
                nc.gpsimd.dma_start(i_out[sg], si)
    return nc


_CACHE = {}


def _get_compiled(tpc):
    if tpc not in _CACHE:
        nc = build_nc(tpc)
        nc.compile()
        _CACHE[tpc] = nc
    return _CACHE[tpc]


def _prep_shared(weight, bias):
    f16 = np.float16
    f8 = ml_dtypes.float8_e4m3
    w = np.asarray(weight, np.float32)
    w_hi = w.astype(f16)
    w_lo22 = ((w - w_hi.astype(np.float32)) * (2.0 ** 22)).astype(f16)
    w8 = (w * 64.0).astype(f8)

    def wtile(a):  # [E, DIM] -> [128, KC, E]
        return np.ascontiguousarray(
            np.ascontiguousarray(a.T).reshape(KC, 128, E).transpose(1, 0, 2)
        )

    wa = np.empty((128, KC, 128), dtype=f16)
    wa[:, :, 0:64] = wtile(w_hi)
    wa[:, :, 64:128] = wtile(w_lo22)

    return {
        "wa": wa,
        "w8": wtile(w8),
        "ident": np.eye(64, dtype=np.float32),
        "identlo": (np.eye(64, dtype=np.float32) * LO_SCALE).astype(
            ml_dtypes.bfloat16
        ),
        "bias_b": np.ascontiguousarray(
            np.broadcast_to(np.asarray(bias, np.float32), (128, E))
        ),
        "iota64": np.ascontiguousarray(
            np.broadcast_to(np.arange(E, dtype=np.float32), (128, E))
        ),
    }


def prep_core_inputs(x, weight, bias, ncores=NCORES, sgt=SGT):
    f16 = np.float16
    f8 = ml_dtypes.float8_e4m3
    shared = _prep_shared(weight, bias)
    x = np.asarray(x, np.float32)
    tpc = x.shape[0] // ncores
    nsg = tpc // sgt
    in_maps = []
    for c in range(ncores):
        xs = np.ascontiguousarray(x[c * tpc:(c + 1) * tpc].T)  # [DIM, tpc]
        xhi = xs.astype(f16)
        r = xs - xhi.astype(np.float32)
        lo8 = (r * 65536.0).astype(f8)
        # hi pack [nsg, KC//KQH, 128, KQH, sgt]: per (sg, half, partition)
        # the [KQH, sgt] block is one 8KB contiguous run in DRAM
        xh6 = xhi.reshape(KC // KQH, KQH, 128, nsg, sgt)
        xh_pack = np.ascontiguousarray(xh6.transpose(3, 0, 2, 1, 4))
        # lo pack [nsg, 128, KC, sgt]: 8KB contiguous per partition
        xl6 = lo8.reshape(KC, 128, nsg, sgt)
        xl_pack = np.ascontiguousarray(xl6.transpose(2, 1, 0, 3))
        in_maps.append({"xh": xh_pack, "xl": xl_pack, **shared})
    return in_maps


def unpack_outputs(res_list, tpc):
    ws, idxs = [], []
    for r in res_list:
        wv = np.asarray(r["w_out"])  # [nsg, 128, cols]
        iv = np.asarray(r["i_out"])
        nsg = wv.shape[0]
        wv = wv.reshape(nsg, 128, -1, TOPK).transpose(0, 2, 1, 3).reshape(tpc, TOPK)
        iv = iv.reshape(nsg, 128, -1, TOPK).transpose(0, 2, 1, 3).reshape(tpc, TOPK)
        ws.append(wv)
        idxs.append(iv)
    return (
        np.ascontiguousarray(np.concatenate(ws)).astype(np.float32),
        np.ascontiguousarray(np.concatenate(idxs)).astype(np.int32),
    )


def run(x, weight, bias, trace=False, **kwargs):
    x = np.asarray(x, np.float32)
    tpc = x.shape[0] // NCORES
    nc = _get_compiled(tpc)
    in_maps = prep_core_inputs(x, weight, bias)
    res = run_bass_kernel_spmd(nc, in_maps, list(range(NCORES)), trace=trace, **kwargs)
    w, i = unpack_outputs(res.results, tpc)
    return w, i, res


def kernel(x, weight, bias):
    w, i, _ = run(x, weight, bias, trace=False)
    return w, i


# revision 16
# speedup vs baseline: 1.0051x; 1.0051x over previous
"""MoE routing gate kernel for Trainium2 (8 NeuronCores, data-parallel).

Computes, for x[32768, 2048], weight[64, 2048], bias[64]:
    logits = x @ weight.T
    probs  = softmax(logits, axis=-1)
    idx    = top_k(probs + bias, 6).indices
    w      = take_along_axis(probs, idx)
returning (w float32 [32768, 6], idx int32 [32768, 6]).

Sharding: tokens split 4096/core across 8 cores; weight/bias replicated.

Per-core pipeline (memory-bound). HBM traffic is the lever: instead of
shipping x at 4 B/elem (fp32 or bf16 hi+lo), the host re-encodes x as
fp16 hi (2 B) + a scaled fp8e4m3 residual (1 B) -- 25.2 MB/shard, a
~70.5 us DMA floor at 360 GB/s vs ~94 us for 4 B/elem.  The logits are
reconstructed on-device to ~4e-6 rms (max ~4e-5) absolute error:

  - Stationary for the fp16 pass packs BOTH weight precision levels in
    the PE array's 128 columns: cols 0-63 = fp16(w), cols 64-127 =
    (w - fp16(w)) * 2^22 in fp16 (the scale keeps the tiny residual out
    of fp16's denormal range).  One pass over x_hi yields w_hi @ x_hi in
    PSUM partitions 0-63 and 2^22 * w_lo @ x_hi in partitions 64-127.
  - The x residual pass is fp8e4m3 x fp8e4m3: lo8 = fp8(r * 2^16) with
    w8 = fp8(w * 2^6); the product scale (2^22) matches the lo partition
    group, so it accumulates straight into partitions 64-127.
  - One PE transpose per 128-token tile (identity = eye(128)) moves both
    groups back to token-major; a single DVE op fuses the recombine
    logit = hi + 2^-22 * lo.
  - Softmax without max-subtraction (|logits| < ~7): ACT exp emits the
    row sum via accum_out.  Selection key q = exp + sum*bias ranks
    identically to probs + bias.
  - DVE Max8/MaxIndex8 give top-8 values+indices; the top-6 unbiased
    weights come from 6 fused scalar_tensor_tensor gathers
    ((iota == idx_k) * exp with accum_out), split across DVE and the
    otherwise-idle Pool engine, then one scale by 1/sum.

x_hi DMAs ride the SP queue, x_lo the ACT queue (an issuing engine is
occupied for its transfer, so the two streams interleave at the DMA
device and the per-DMA fixed costs hide under each other).
"""

import numpy as np
import ml_dtypes

import concourse.bacc as bacc
import concourse.bass as bass
import concourse.mybir as mybir
import concourse.tile as tile
from concourse.bass_utils import run_bass_kernel_spmd

F16 = mybir.dt.float16
F8 = mybir.dt.float8e4
BF16 = mybir.dt.bfloat16
F32 = mybir.dt.float32
I32 = mybir.dt.int32
U16 = mybir.dt.uint16
OP = mybir.AluOpType
EXP = mybir.ActivationFunctionType.Exp

TOKENS, DIM, E, TOPK, NCORES = 32768, 2048, 64, 6, 8
KC = DIM // 128   # contraction chunks of 128
SGT = 512         # tokens per super-group (= matmul N, one PSUM bank)
KQH = 8           # fp16 k-chunks per DMA (2 DMAs per sg)
LO_SCALE = float(2.0 ** -22)


def build_nc(tpc, sgt=SGT):
    """Build the per-core Bass program for a tpc-token shard."""
    nsg = tpc // sgt
    nj = sgt // 128       # 128-token tiles per super-group
    cols = nj * TOPK      # staging cols per sg

    nc = bacc.Bacc("TRN2", target_bir_lowering=False, debug=False)

    xh = nc.dram_tensor("xh", [nsg, KC // KQH, 128, KQH, sgt], F16, kind="ExternalInput")
    xl = nc.dram_tensor("xl", [nsg, 128, KC, sgt], F8, kind="ExternalInput")
    wa = nc.dram_tensor("wa", [128, KC, 128], F16, kind="ExternalInput")
    w8 = nc.dram_tensor("w8", [128, KC, E], F8, kind="ExternalInput")
    # eye(64) fp32 for the hi transpose; eye(64)*2^-22 in bf16 (exact: power
    # of two) for the lo-group recombine matmul.  The recombine must NOT be
    # an fp32 matmul: a regular fp32 matmul following FWL-loaded f16/f8
    # matmuls hangs the PE (the LastMatmultFP32HI erratum); bf16 is safe.
    ident = nc.dram_tensor("ident", [64, 64], F32, kind="ExternalInput")
    identlo = nc.dram_tensor("identlo", [64, 64], BF16, kind="ExternalInput")
    bias_b = nc.dram_tensor("bias_b", [128, E], F32, kind="ExternalInput")
    iota64 = nc.dram_tensor("iota64", [128, E], F32, kind="ExternalInput")
    w_out = nc.dram_tensor("w_out", [nsg, 128, cols], F32, kind="ExternalOutput")
    i_out = nc.dram_tensor("i_out", [nsg, 128, cols], I32, kind="ExternalOutput")

    with tile.TileContext(nc) as tc:
        with (
            tc.tile_pool(name="consts", bufs=1) as cpool,
            tc.tile_pool(name="xhbuf", bufs=5) as xhp,
            tc.tile_pool(name="xlbuf", bufs=3) as xlp,
            tc.tile_pool(name="lt", bufs=2) as ltp,
            tc.tile_pool(name="small", bufs=8) as smp,
            tc.tile_pool(name="work", bufs=4) as wkp,
            tc.tile_pool(name="stage", bufs=3) as stp,
            tc.tile_pool(name="acc", bufs=2, space="PSUM") as accp,
            tc.tile_pool(name="tr", bufs=4, space="PSUM") as trp,
        ):
            cwa = cpool.tile([128, KC, 128], F16)
            nc.gpsimd.dma_start(cwa, wa[:])
            cw8 = cpool.tile([128, KC, E], F8)
            nc.gpsimd.dma_start(cw8, w8[:])
            cident = cpool.tile([64, 64], F32)
            nc.gpsimd.dma_start(cident, ident[:])
            cidlo = cpool.tile([64, 64], BF16)
            nc.gpsimd.dma_start(cidlo, identlo[:])
            cbias = cpool.tile([128, E], F32)
            nc.gpsimd.dma_start(cbias, bias_b[:])
            ciota = cpool.tile([128, E], F32)
            nc.gpsimd.dma_start(ciota, iota64[:])

            for sg in range(nsg):
                # second hi chunk split 6+2 so the last-arriving DMA is
                # small: the PE tail after the final byte is 2 k-chunks
                xh0 = xhp.tile([128, KQH, sgt], F16, tag="xh0")
                nc.sync.dma_start(xh0, xh[sg, 0])
                xlt = xlp.tile([128, KC, sgt], F8, tag="xl")
                nc.scalar.dma_start(xlt, xl[sg])
                xh1 = xhp.tile([128, KQH - 2, sgt], F16, tag="xh1")
                nc.sync.dma_start(xh1, xh[sg, 1, :, 0:KQH - 2])
                xh2 = xhp.tile([128, 2, sgt], F16, tag="xh2")
                nc.sync.dma_start(xh2, xh[sg, 1, :, KQH - 2:KQH])

                acc = accp.tile([128, sgt], F32)
                for k in range(KQH):
                    nc.tensor.matmul(
                        acc, cwa[:, k, :], xh0[:, k, :],
                        start=(k == 0), stop=False,
                    )
                for k in range(KC):
                    nc.tensor.matmul(
                        acc[64:128], cw8[:, k, :], xlt[:, k, :],
                        start=False, stop=False,
                    )
                for k in range(KQH, KC - 2):
                    nc.tensor.matmul(
                        acc, cwa[:, k, :], xh1[:, k - KQH, :],
                        start=False, stop=False,
                    )
                for k in range(KC - 2, KC):
                    nc.tensor.matmul(
                        acc, cwa[:, k, :], xh2[:, k - (KC - 2), :],
                        start=False, stop=(k == KC - 1),
                    )

                lth = ltp.tile([64, sgt], F32, tag="lth")
                nc.scalar.copy(lth, acc[0:64])
                ltl = ltp.tile([64, sgt], BF16, tag="ltl")
                nc.scalar.copy(ltl, acc[64:128])

                sw = stp.tile([128, cols], F32, tag="sw")
                si = stp.tile([128, cols], I32, tag="si")
                for j in range(nj):
                    chunk = slice(j * 128, (j + 1) * 128)
                    tps = trp.tile([128, E], F32)
                    nc.tensor.matmul(
                        tps, lth[:, chunk], cident,
                        is_transpose=True, start=True, stop=False,
                    )
                    nc.tensor.matmul(
                        tps, ltl[:, chunk], cidlo,
                        start=False, stop=True,
                    )
                    ex = wkp.tile([128, E], F32, tag="ex", bufs=nj + 2)
                    ssum = smp.tile([128, 1], F32, tag="ssum")
                    nc.scalar.activation(ex, tps, EXP, accum_out=ssum)
                    q = wkp.tile([128, E], F32, tag="q")
                    nc.vector.scalar_tensor_tensor(
                        q, cbias, ssum, ex, OP.mult, OP.add
                    )
                    mx = smp.tile([128, 8], F32, tag="mx")
                    nc.vector.max(mx, q)
                    mi = smp.tile([128, 8], U16, tag="mi")
                    nc.vector.max_index(mi, mx, q)
                    idxf = smp.tile([128, 8], F32, tag="idxf")
                    nc.gpsimd.tensor_copy(idxf, mi)
                    rs = smp.tile([128, 1], F32, tag="rs")
                    nc.vector.reciprocal(rs, ssum)
                    col = j * TOPK
                    nc.gpsimd.tensor_copy(si[:, col:col + TOPK], mi[:, 0:TOPK])
                    scr = wkp.tile([128, TOPK, E], F32, tag="scr")
                    g6 = smp.tile([128, TOPK], F32, tag="g6")
                    for kk in range(TOPK):
                        nc.vector.scalar_tensor_tensor(
                            scr[:, kk], ciota, idxf[:, kk:kk + 1], ex,
                            OP.is_equal, OP.mult,
                            accum_out=g6[:, kk:kk + 1],
                        )
                    nc.vector.tensor_scalar_mul(sw[:, col:col + TOPK], g6, rs)
                nc.gpsimd.dma_start(w_out[sg], sw)
                nc.sync.dma_start(i_out[sg], si)
    return nc


_CACHE = {}


def _get_compiled(tpc):
    if tpc not in _CACHE:
        nc = build_nc(tpc)
        nc.compile()
        _CACHE[tpc] = nc
    return _CACHE[tpc]


def _prep_shared(weight, bias):
    f16 = np.float16
    f8 = ml_dtypes.float8_e4m3
    w = np.asarray(weight, np.float32)
    w_hi = w.astype(f16)
    w_lo22 = ((w - w_hi.astype(np.float32)) * (2.0 ** 22)).astype(f16)
    w8 = (w * 64.0).astype(f8)

    def wtile(a):  # [E, DIM] -> [128, KC, E]
        return np.ascontiguousarray(
            np.ascontiguousarray(a.T).reshape(KC, 128, E).transpose(1, 0, 2)
        )

    wa = np.empty((128, KC, 128), dtype=f16)
    wa[:, :, 0:64] = wtile(w_hi)
    wa[:, :, 64:128] = wtile(w_lo22)

    return {
        "wa": wa,
        "w8": wtile(w8),
        "ident": np.eye(64, dtype=np.float32),
        "identlo": (np.eye(64, dtype=np.float32) * LO_SCALE).astype(
            ml_dtypes.bfloat16
        ),
        "bias_b": np.ascontiguousarray(
            np.broadcast_to(np.asarray(bias, np.float32), (128, E))
        ),
        "iota64": np.ascontiguousarray(
            np.broadcast_to(np.arange(E, dtype=np.float32), (128, E))
        ),
    }


def prep_core_inputs(x, weight, bias, ncores=NCORES, sgt=SGT):
    f16 = np.float16
    f8 = ml_dtypes.float8_e4m3
    shared = _prep_shared(weight, bias)
    x = np.asarray(x, np.float32)
    tpc = x.shape[0] // ncores
    nsg = tpc // sgt
    in_maps = []
    for c in range(ncores):
        xs = np.ascontiguousarray(x[c * tpc:(c + 1) * tpc].T)  # [DIM, tpc]
        xhi = xs.astype(f16)
        r = xs - xhi.astype(np.float32)
        lo8 = (r * 65536.0).astype(f8)
        # hi pack [nsg, KC//KQH, 128, KQH, sgt]: per (sg, half, partition)
        # the [KQH, sgt] block is one 8KB contiguous run in DRAM
        xh6 = xhi.reshape(KC // KQH, KQH, 128, nsg, sgt)
        xh_pack = np.ascontiguousarray(xh6.transpose(3, 0, 2, 1, 4))
        # lo pack [nsg, 128, KC, sgt]: 8KB contiguous per partition
        xl6 = lo8.reshape(KC, 128, nsg, sgt)
        xl_pack = np.ascontiguousarray(xl6.transpose(2, 1, 0, 3))
        in_maps.append({"xh": xh_pack, "xl": xl_pack, **shared})
    return in_maps


def unpack_outputs(res_list, tpc):
    ws, idxs = [], []
    for r in res_list:
        wv = np.asarray(r["w_out"])  # [nsg, 128, cols]
        iv = np.asarray(r["i_out"])
        nsg = wv.shape[0]
        wv = wv.reshape(nsg, 128, -1, TOPK).transpose(0, 2, 1, 3).reshape(tpc, TOPK)
        iv = iv.reshape(nsg, 128, -1, TOPK).transpose(0, 2, 1, 3).reshape(tpc, TOPK)
        ws.append(wv)
        idxs.append(iv)
    return (
        np.ascontiguousarray(np.concatenate(ws)).astype(np.float32),
        np.ascontiguousarray(np.concatenate(idxs)).astype(np.int32),
    )


def run(x, weight, bias, trace=False, **kwargs):
    x = np.asarray(x, np.float32)
    tpc = x.shape[0] // NCORES
    nc = _get_compiled(tpc)
    in_maps = prep_core_inputs(x, weight, bias)
    res = run_bass_kernel_spmd(nc, in_maps, list(range(NCORES)), trace=trace, **kwargs)
    w, i = unpack_outputs(res.results, tpc)
    return w, i, res


def kernel(x, weight, bias):
    w, i, _ = run(x, weight, bias, trace=False)
    return w, i


# revision 21
# speedup vs baseline: 1.0158x; 1.0106x over previous
"""MoE routing gate kernel for Trainium2 (8 NeuronCores, data-parallel).

Computes, for x[32768, 2048], weight[64, 2048], bias[64]:
    logits = x @ weight.T
    probs  = softmax(logits, axis=-1)
    idx    = top_k(probs + bias, 6).indices
    w      = take_along_axis(probs, idx)
returning (w float32 [32768, 6], idx int32 [32768, 6]).

Sharding: tokens split 4096/core across 8 cores; weight/bias replicated.

Per-core pipeline (memory-bound). HBM traffic is the lever: instead of
shipping x at 4 B/elem (fp32 or bf16 hi+lo), the host re-encodes x as
fp16 hi (2 B) + a scaled fp8e4m3 residual (1 B) -- 25.2 MB/shard, a
~70 us DMA floor at 360 GB/s vs ~94 us for 4 B/elem.  The logits are
reconstructed on-device to ~4e-6 rms (max ~4e-5) absolute error:

  - Stationary for the fp16 pass packs BOTH weight precision levels in
    the PE array's 128 columns: cols 0-63 = fp16(w), cols 64-127 =
    (w - fp16(w)) * 2^22 in fp16 (the scale keeps the tiny residual out
    of fp16's denormal range).  One pass over x_hi yields w_hi @ x_hi in
    PSUM partitions 0-63 and 2^22 * w_lo @ x_hi in partitions 64-127.
  - The x residual pass is fp8e4m3 x fp8e4m3: lo8 = fp8(r * 2^16) with
    w8 = fp8(w * 2^6); the product scale (2^22) matches the lo partition
    group, so it accumulates straight into partitions 64-127.
  - Per 128-token tile, a PE transpose (identity, fp32) moves the hi
    group back to token-major and a bf16 matmul against eye*2^-22 folds
    the rescaled lo group into the same PSUM accumulation.  (The
    recombine must NOT be an fp32 matmul: a regular fp32 matmul after
    FWL-loaded f16/f8 matmuls hangs the PE -- LastMatmultFP32HI erratum;
    bf16 is exact here since eye*2^-22 is a power of two and the 2^-9
    rounding applies to a term that is ~2^-11 of the logit.)
  - Softmax without max-subtraction (|logits| < ~7): ACT exp emits the
    row sum via accum_out.  Selection key q = exp + sum*bias ranks
    identically to probs + bias.
  - DVE Max8/MaxIndex8 give top-8 values+indices; the top-6 unbiased
    weights come from 6 fused scalar_tensor_tensor gathers
    ((iota == idx_k) * exp with accum_out), then one scale by 1/sum.
    Index staging copies ride the idle Pool engine.

Schedule: x_hi DMAs on the SP queue, x_lo on the ACT queue (an issuing
engine is occupied for its transfer, so the two streams interleave at
the DMA device and per-DMA fixed costs hide under each other).  Each
group's hi stream is split [8,6,2] k-chunks so the last-arriving DMA is
small.  Token groups are 7x512 + 2x256: the tail after the final byte
only has to route 256 tokens (2 tiles) of top-k DVE work, and the
second 256-group's routing hides under the last group's DMA window.
"""

import numpy as np
import ml_dtypes

import concourse.bacc as bacc
import concourse.bass as bass
import concourse.mybir as mybir
import concourse.tile as tile
from concourse.bass_utils import run_bass_kernel_spmd

F16 = mybir.dt.float16
F8 = mybir.dt.float8e4
BF16 = mybir.dt.bfloat16
F32 = mybir.dt.float32
I32 = mybir.dt.int32
U16 = mybir.dt.uint16
OP = mybir.AluOpType
EXP = mybir.ActivationFunctionType.Exp

TOKENS, DIM, E, TOPK, NCORES = 32768, 2048, 64, 6, 8
KC = DIM // 128     # contraction chunks of 128
LO_SCALE = float(2.0 ** -22)


def sg_layout(tpc):
    """Token-group sizes: 512s, with the final 512 split 256+256 so the
    post-stream routing tail is halved."""
    sgs = [512] * (tpc // 512 - 1) + [256, 256]
    assert sum(sgs) == tpc
    return sgs


def build_nc(tpc):
    """Build the per-core Bass program for a tpc-token shard."""
    sgs = sg_layout(tpc)
    ntile = tpc // 128

    nc = bacc.Bacc("TRN2", target_bir_lowering=False, debug=False)

    # One [128, KC, sgt] tensor per token group, [partition][chunk][token]:
    # a k-range slice is contiguous per partition (4-16 KB runs), which
    # keeps DMA descriptors long enough for full HBM rate.
    xh_t = [
        nc.dram_tensor(f"xh{s}", [128, KC, sgt], F16, kind="ExternalInput")
        for s, sgt in enumerate(sgs)
    ]
    xl_t = [
        nc.dram_tensor(f"xl{s}", [128, KC, sgt], F8, kind="ExternalInput")
        for s, sgt in enumerate(sgs)
    ]
    wa = nc.dram_tensor("wa", [128, KC, 128], F16, kind="ExternalInput")
    w8 = nc.dram_tensor("w8", [128, KC, E], F8, kind="ExternalInput")
    ident = nc.dram_tensor("ident", [64, 64], F32, kind="ExternalInput")
    identlo = nc.dram_tensor("identlo", [64, 64], BF16, kind="ExternalInput")
    bias_b = nc.dram_tensor("bias_b", [128, E], F32, kind="ExternalInput")
    iota64 = nc.dram_tensor("iota64", [128, E], F32, kind="ExternalInput")
    # outputs token-tile-major: tile t = tokens [t*128, (t+1)*128)
    w_out = nc.dram_tensor("w_out", [128, ntile, TOPK], F32, kind="ExternalOutput")
    i_out = nc.dram_tensor("i_out", [128, ntile, TOPK], I32, kind="ExternalOutput")

    with tile.TileContext(nc) as tc:
        with (
            tc.tile_pool(name="consts", bufs=1) as cpool,
            tc.tile_pool(name="xhbuf", bufs=3) as xhp,
            tc.tile_pool(name="xlbuf", bufs=3) as xlp,
            tc.tile_pool(name="lt", bufs=2) as ltp,
            tc.tile_pool(name="small", bufs=8) as smp,
            tc.tile_pool(name="work", bufs=4) as wkp,
            tc.tile_pool(name="stage", bufs=3) as stp,
            tc.tile_pool(name="acc", bufs=2, space="PSUM") as accp,
            tc.tile_pool(name="tr", bufs=4, space="PSUM") as trp,
        ):
            cwa = cpool.tile([128, KC, 128], F16)
            nc.gpsimd.dma_start(cwa, wa[:])
            cw8 = cpool.tile([128, KC, E], F8)
            nc.gpsimd.dma_start(cw8, w8[:])
            cident = cpool.tile([64, 64], F32)
            nc.gpsimd.dma_start(cident, ident[:])
            cidlo = cpool.tile([64, 64], BF16)
            nc.gpsimd.dma_start(cidlo, identlo[:])
            cbias = cpool.tile([128, E], F32)
            nc.gpsimd.dma_start(cbias, bias_b[:])
            ciota = cpool.tile([128, E], F32)
            nc.gpsimd.dma_start(ciota, iota64[:])

            base = 0
            for sg, sgt in enumerate(sgs):
                nj = sgt // 128
                # hi k-chunks split [8, 6, 2]: the last-arriving DMA is
                # small, so the PE tail after the final byte is short
                xh0 = xhp.tile([128, 8, sgt], F16, tag=f"xh0_{sgt}")
                nc.sync.dma_start(xh0, xh_t[sg][:, 0:8, :])
                xlt = xlp.tile([128, KC, sgt], F8, tag=f"xl_{sgt}")
                nc.scalar.dma_start(xlt, xl_t[sg][:])
                xh1 = xhp.tile([128, 6, sgt], F16, tag=f"xh1_{sgt}")
                nc.sync.dma_start(xh1, xh_t[sg][:, 8:14, :])
                xh2 = xhp.tile([128, 2, sgt], F16, tag=f"xh2_{sgt}")
                nc.sync.dma_start(xh2, xh_t[sg][:, 14:16, :])

                acc = accp.tile([128, sgt], F32, tag=f"acc_{sgt}")
                for k in range(8):
                    nc.tensor.matmul(
                        acc, cwa[:, k, :], xh0[:, k, :],
                        start=(k == 0), stop=False,
                    )
                for k in range(KC):
                    nc.tensor.matmul(
                        acc[64:128], cw8[:, k, :], xlt[:, k, :],
                        start=False, stop=False,
                    )
                for k in range(8, 14):
                    nc.tensor.matmul(
                        acc, cwa[:, k, :], xh1[:, k - 8, :],
                        start=False, stop=False,
                    )
                for k in range(14, KC):
                    nc.tensor.matmul(
                        acc, cwa[:, k, :], xh2[:, k - 14, :],
                        start=False, stop=(k == KC - 1),
                    )

                lth = ltp.tile([64, sgt], F32, tag=f"lth_{sgt}")
                nc.scalar.copy(lth, acc[0:64])
                ltl = ltp.tile([64, sgt], BF16, tag=f"ltl_{sgt}")
                nc.scalar.copy(ltl, acc[64:128])

                cols = nj * TOPK
                sw = stp.tile([128, cols], F32, tag=f"sw_{sgt}")
                si = stp.tile([128, cols], I32, tag=f"si_{sgt}")
                for j in range(nj):
                    chunk = slice(j * 128, (j + 1) * 128)
                    tps = trp.tile([128, E], F32)
                    nc.tensor.matmul(
                        tps, lth[:, chunk], cident,
                        is_transpose=True, start=True, stop=False,
                    )
                    nc.tensor.matmul(
                        tps, ltl[:, chunk], cidlo,
                        start=False, stop=True,
                    )
                    ex = wkp.tile([128, E], F32, tag="ex", bufs=6)
                    ssum = smp.tile([128, 1], F32, tag="ssum")
                    nc.scalar.activation(ex, tps, EXP, accum_out=ssum)
                    q = wkp.tile([128, E], F32, tag="q")
                    nc.vector.scalar_tensor_tensor(
                        q, cbias, ssum, ex, OP.mult, OP.add
                    )
                    mx = smp.tile([128, 8], F32, tag="mx")
                    nc.vector.max(mx, q)
                    mi = smp.tile([128, 8], U16, tag="mi")
                    nc.vector.max_index(mi, mx, q)
                    idxf = smp.tile([128, 8], F32, tag="idxf")
                    nc.gpsimd.tensor_copy(idxf, mi)
                    rs = smp.tile([128, 1], F32, tag="rs")
                    nc.vector.reciprocal(rs, ssum)
                    col = j * TOPK
                    nc.gpsimd.tensor_copy(si[:, col:col + TOPK], mi[:, 0:TOPK])
                    scr = wkp.tile([128, TOPK, E], F32, tag="scr")
                    g6 = smp.tile([128, TOPK], F32, tag="g6")
                    for kk in range(TOPK):
                        nc.vector.scalar_tensor_tensor(
                            scr[:, kk], ciota, idxf[:, kk:kk + 1], ex,
                            OP.is_equal, OP.mult,
                            accum_out=g6[:, kk:kk + 1],
                        )
                    nc.vector.tensor_scalar_mul(sw[:, col:col + TOPK], g6, rs)
                tb = base // 128
                nc.gpsimd.dma_start(w_out[:, tb:tb + nj, :], sw)
                nc.sync.dma_start(i_out[:, tb:tb + nj, :], si)
                base += sgt
    return nc


_CACHE = {}


def _get_compiled(tpc):
    if tpc not in _CACHE:
        nc = build_nc(tpc)
        nc.compile()
        _CACHE[tpc] = nc
    return _CACHE[tpc]


def _prep_shared(weight, bias):
    f16 = np.float16
    f8 = ml_dtypes.float8_e4m3
    w = np.asarray(weight, np.float32)
    w_hi = w.astype(f16)
    w_lo22 = ((w - w_hi.astype(np.float32)) * (2.0 ** 22)).astype(f16)
    w8 = (w * 64.0).astype(f8)

    def wtile(a):  # [E, DIM] -> [128, KC, E]
        return np.ascontiguousarray(
            np.ascontiguousarray(a.T).reshape(KC, 128, E).transpose(1, 0, 2)
        )

    wa = np.empty((128, KC, 128), dtype=f16)
    wa[:, :, 0:64] = wtile(w_hi)
    wa[:, :, 64:128] = wtile(w_lo22)

    return {
        "wa": wa,
        "w8": wtile(w8),
        "ident": np.eye(64, dtype=np.float32),
        "identlo": (np.eye(64, dtype=np.float32) * LO_SCALE).astype(
            ml_dtypes.bfloat16
        ),
        "bias_b": np.ascontiguousarray(
            np.broadcast_to(np.asarray(bias, np.float32), (128, E))
        ),
        "iota64": np.ascontiguousarray(
            np.broadcast_to(np.arange(E, dtype=np.float32), (128, E))
        ),
    }


def prep_core_inputs(x, weight, bias, ncores=NCORES):
    f16 = np.float16
    f8 = ml_dtypes.float8_e4m3
    shared = _prep_shared(weight, bias)
    x = np.asarray(x, np.float32)
    tpc = x.shape[0] // ncores
    in_maps = []
    sgs = sg_layout(tpc)
    for c in range(ncores):
        xs = np.ascontiguousarray(x[c * tpc:(c + 1) * tpc].T)  # [DIM, tpc]
        xhi = xs.astype(f16)
        r = xs - xhi.astype(np.float32)
        lo8 = (r * 65536.0).astype(f8)
        # [DIM, tpc] -> per-group [128, KC, sgt]
        xh_pack = xhi.reshape(KC, 128, tpc).transpose(1, 0, 2)
        xl_pack = lo8.reshape(KC, 128, tpc).transpose(1, 0, 2)
        m = {**shared}
        base = 0
        for s, sgt in enumerate(sgs):
            m[f"xh{s}"] = np.ascontiguousarray(xh_pack[:, :, base:base + sgt])
            m[f"xl{s}"] = np.ascontiguousarray(xl_pack[:, :, base:base + sgt])
            base += sgt
        in_maps.append(m)
    return in_maps


def unpack_outputs(res_list, tpc):
    ws, idxs = [], []
    for r in res_list:
        wv = np.asarray(r["w_out"])  # [128, ntile, TOPK]
        iv = np.asarray(r["i_out"])
        ws.append(wv.transpose(1, 0, 2).reshape(tpc, TOPK))
        idxs.append(iv.transpose(1, 0, 2).reshape(tpc, TOPK))
    return (
        np.ascontiguousarray(np.concatenate(ws)).astype(np.float32),
        np.ascontiguousarray(np.concatenate(idxs)).astype(np.int32),
    )


def run(x, weight, bias, trace=False, **kwargs):
    x = np.asarray(x, np.float32)
    tpc = x.shape[0] // NCORES
    nc = _get_compiled(tpc)
    in_maps = prep_core_inputs(x, weight, bias)
    res = run_bass_kernel_spmd(nc, in_maps, list(range(NCORES)), trace=trace, **kwargs)
    w, i = unpack_outputs(res.results, tpc)
    return w, i, res


def kernel(x, weight, bias):
    w, i, _ = run(x, weight, bias, trace=False)
    return w, i


# revision 24
# speedup vs baseline: 1.0342x; 1.0181x over previous
"""MoE routing gate kernel for Trainium2 (8 NeuronCores, data-parallel).

Computes, for x[32768, 2048], weight[64, 2048], bias[64]:
    logits = x @ weight.T
    probs  = softmax(logits, axis=-1)
    idx    = top_k(probs + bias, 6).indices
    w      = take_along_axis(probs, idx)
returning (w float32 [32768, 6], idx int32 [32768, 6]).

Sharding: tokens split 4096/core across 8 cores; weight/bias replicated.

Per-core pipeline (memory-bound). HBM traffic is the lever: instead of
shipping x at 4 B/elem (fp32 or bf16 hi+lo), the host re-encodes x as
fp16 hi (2 B) + a scaled fp8e4m3 residual (1 B) -- 25.2 MB/shard, a
~70 us DMA floor at 360 GB/s vs ~94 us for 4 B/elem.  The logits are
reconstructed on-device to ~4e-6 rms (max ~4e-5) absolute error:

  - Stationary for the fp16 pass packs BOTH weight precision levels in
    the PE array's 128 columns: cols 0-63 = fp16(w), cols 64-127 =
    (w - fp16(w)) * 2^22 in fp16 (the scale keeps the tiny residual out
    of fp16's denormal range).  One pass over x_hi yields w_hi @ x_hi in
    PSUM partitions 0-63 and 2^22 * w_lo @ x_hi in partitions 64-127.
  - The x residual pass is fp8e4m3 x fp8e4m3: lo8 = fp8(r * 2^16) with
    w8 = fp8(w * 2^6); the product scale (2^22) matches the lo partition
    group, so it accumulates straight into partitions 64-127.
  - Per 128-token tile, a PE transpose (identity, fp32) moves the hi
    group back to token-major and a bf16 matmul against eye*2^-22 folds
    the rescaled lo group into the same PSUM accumulation.  (The
    recombine must NOT be an fp32 matmul: a regular fp32 matmul after
    FWL-loaded f16/f8 matmuls hangs the PE -- LastMatmultFP32HI erratum;
    bf16 is exact here since eye*2^-22 is a power of two and the 2^-9
    rounding applies to a term that is ~2^-11 of the logit.)
  - Softmax without max-subtraction (|logits| < ~7): ACT exp emits the
    row sum via accum_out.  Selection key q = exp + sum*bias ranks
    identically to probs + bias.
  - DVE Max8/MaxIndex8 give top-8 values+indices; the top-6 unbiased
    weights come from 6 fused scalar_tensor_tensor gathers
    ((iota == idx_k) * exp with accum_out), then one scale by 1/sum.
    Index staging copies ride the idle Pool engine.

Schedule: x_hi DMAs on the SP queue, x_lo on the ACT queue (an issuing
engine is occupied for its transfer, so the two streams interleave at
the DMA device and per-DMA fixed costs hide under each other).  Each
group's hi stream is split [8,6,2] k-chunks so the last-arriving DMA is
small.  Token groups are 7x512 + 2x256: the tail after the final byte
only has to route 256 tokens (2 tiles) of top-k DVE work, and the
second 256-group's routing hides under the last group's DMA window.
"""

import numpy as np
import ml_dtypes

import concourse.bacc as bacc
import concourse.bass as bass
import concourse.mybir as mybir
import concourse.tile as tile
from concourse.bass_utils import run_bass_kernel_spmd

F16 = mybir.dt.float16
F8 = mybir.dt.float8e4
BF16 = mybir.dt.bfloat16
F32 = mybir.dt.float32
I32 = mybir.dt.int32
U16 = mybir.dt.uint16
OP = mybir.AluOpType
EXP = mybir.ActivationFunctionType.Exp

TOKENS, DIM, E, TOPK, NCORES = 32768, 2048, 64, 6, 8
KC = DIM // 128     # contraction chunks of 128
LO_SCALE = float(2.0 ** -22)


def sg_layout(tpc):
    """Token-group sizes: 512s, with the final 512 split 256+256 so the
    post-stream routing tail is halved."""
    sgs = [512] * (tpc // 512 - 1) + [256, 256]
    assert sum(sgs) == tpc
    return sgs


def build_nc(tpc):
    """Build the per-core Bass program for a tpc-token shard."""
    sgs = sg_layout(tpc)
    ntile = tpc // 128

    nc = bacc.Bacc("TRN2", target_bir_lowering=False, debug=False)

    # One [128, KC, sgt] tensor per token group, [partition][chunk][token]:
    # a k-range slice is contiguous per partition (4-16 KB runs), which
    # keeps DMA descriptors long enough for full HBM rate.
    xh_t = [
        nc.dram_tensor(f"xh{s}", [128, KC, sgt], F16, kind="ExternalInput")
        for s, sgt in enumerate(sgs)
    ]
    xl_t = [
        nc.dram_tensor(f"xl{s}", [128, KC, sgt], F8, kind="ExternalInput")
        for s, sgt in enumerate(sgs)
    ]
    wa = nc.dram_tensor("wa", [128, KC, 128], F16, kind="ExternalInput")
    w8 = nc.dram_tensor("w8", [128, KC, E], F8, kind="ExternalInput")
    ident = nc.dram_tensor("ident", [64, 64], F32, kind="ExternalInput")
    identlo = nc.dram_tensor("identlo", [64, 64], BF16, kind="ExternalInput")
    bias_b = nc.dram_tensor("bias_b", [128, E], F32, kind="ExternalInput")
    iota64 = nc.dram_tensor("iota64", [128, E], F32, kind="ExternalInput")
    # outputs token-tile-major: tile t = tokens [t*128, (t+1)*128)
    w_out = nc.dram_tensor("w_out", [128, ntile, TOPK], F32, kind="ExternalOutput")
    i_out = nc.dram_tensor("i_out", [128, ntile, TOPK], I32, kind="ExternalOutput")

    with tile.TileContext(nc) as tc:
        with (
            tc.tile_pool(name="consts", bufs=1) as cpool,
            tc.tile_pool(name="xhbuf", bufs=3) as xhp,
            tc.tile_pool(name="xlbuf", bufs=3) as xlp,
            tc.tile_pool(name="lt", bufs=2) as ltp,
            tc.tile_pool(name="small", bufs=8) as smp,
            tc.tile_pool(name="work", bufs=4) as wkp,
            tc.tile_pool(name="stage", bufs=3) as stp,
            tc.tile_pool(name="acc", bufs=2, space="PSUM") as accp,
            tc.tile_pool(name="tr", bufs=4, space="PSUM") as trp,
        ):
            cwa = cpool.tile([128, KC, 128], F16)
            nc.gpsimd.dma_start(cwa, wa[:])
            cw8 = cpool.tile([128, KC, E], F8)
            nc.gpsimd.dma_start(cw8, w8[:])
            cident = cpool.tile([64, 64], F32)
            nc.gpsimd.dma_start(cident, ident[:])
            cidlo = cpool.tile([64, 64], BF16)
            nc.gpsimd.dma_start(cidlo, identlo[:])
            cbias = cpool.tile([128, E], F32)
            nc.gpsimd.dma_start(cbias, bias_b[:])
            ciota = cpool.tile([128, E], F32)
            nc.gpsimd.dma_start(ciota, iota64[:])

            base = 0
            for sg, sgt in enumerate(sgs):
                nj = sgt // 128
                # hi k-chunks split [8, 6, 2]: the last-arriving DMA is
                # small, so the PE tail after the final byte is short
                xh0 = xhp.tile([128, 8, sgt], F16, tag=f"xh0_{sgt}")
                nc.sync.dma_start(xh0, xh_t[sg][:, 0:8, :])
                xlt = xlp.tile([128, KC, sgt], F8, tag=f"xl_{sgt}")
                # the last groups' lo DMAs ride SP so ACT is free for the
                # tail's copy/exp chain as soon as the matmuls finish
                xl_eng = nc.sync if sg >= len(sgs) - 2 else nc.scalar
                xl_eng.dma_start(xlt, xl_t[sg][:])
                xh1 = xhp.tile([128, 6, sgt], F16, tag=f"xh1_{sgt}")
                nc.sync.dma_start(xh1, xh_t[sg][:, 8:14, :])
                xh2 = xhp.tile([128, 2, sgt], F16, tag=f"xh2_{sgt}")
                nc.sync.dma_start(xh2, xh_t[sg][:, 14:16, :])

                acc = accp.tile([128, sgt], F32, tag=f"acc_{sgt}")
                for k in range(8):
                    nc.tensor.matmul(
                        acc, cwa[:, k, :], xh0[:, k, :],
                        start=(k == 0), stop=False,
                    )
                for k in range(KC):
                    nc.tensor.matmul(
                        acc[64:128], cw8[:, k, :], xlt[:, k, :],
                        start=False, stop=False,
                    )
                for k in range(8, 14):
                    nc.tensor.matmul(
                        acc, cwa[:, k, :], xh1[:, k - 8, :],
                        start=False, stop=False,
                    )
                for k in range(14, KC):
                    nc.tensor.matmul(
                        acc, cwa[:, k, :], xh2[:, k - 14, :],
                        start=False, stop=(k == KC - 1),
                    )

                lth = ltp.tile([64, sgt], F32, tag=f"lth_{sgt}")
                nc.scalar.copy(lth, acc[0:64])
                ltl = ltp.tile([64, sgt], BF16, tag=f"ltl_{sgt}")
                nc.scalar.copy(ltl, acc[64:128])

                cols = nj * TOPK
                sw = stp.tile([128, cols], F32, tag=f"sw_{sgt}")
                si = stp.tile([128, cols], I32, tag=f"si_{sgt}")
                for j in range(nj):
                    chunk = slice(j * 128, (j + 1) * 128)
                    tps = trp.tile([128, E], F32)
                    nc.tensor.matmul(
                        tps, lth[:, chunk], cident,
                        is_transpose=True, start=True, stop=False,
                    )
                    nc.tensor.matmul(
                        tps, ltl[:, chunk], cidlo,
                        start=False, stop=True,
                    )
                    ex = wkp.tile([128, E], F32, tag="ex", bufs=6)
                    ssum = smp.tile([128, 1], F32, tag="ssum")
                    nc.scalar.activation(ex, tps, EXP, accum_out=ssum)
                    q = wkp.tile([128, E], F32, tag="q")
                    nc.vector.scalar_tensor_tensor(
                        q, cbias, ssum, ex, OP.mult, OP.add
                    )
                    mx = smp.tile([128, 8], F32, tag="mx")
                    nc.vector.max(mx, q)
                    mi = smp.tile([128, 8], U16, tag="mi")
                    nc.vector.max_index(mi, mx, q)
                    idxf = smp.tile([128, 8], F32, tag="idxf")
                    nc.gpsimd.tensor_copy(idxf, mi)
                    rs = smp.tile([128, 1], F32, tag="rs")
                    nc.vector.reciprocal(rs, ssum)
                    col = j * TOPK
                    nc.gpsimd.tensor_copy(si[:, col:col + TOPK], mi[:, 0:TOPK])
                    scr = wkp.tile([128, TOPK, E], F32, tag="scr")
                    g6 = smp.tile([128, TOPK], F32, tag="g6")
                    for kk in range(TOPK):
                        nc.vector.scalar_tensor_tensor(
                            scr[:, kk], ciota, idxf[:, kk:kk + 1], ex,
                            OP.is_equal, OP.mult,
                            accum_out=g6[:, kk:kk + 1],
                        )
                    nc.vector.tensor_scalar_mul(sw[:, col:col + TOPK], g6, rs)
                tb = base // 128
                nc.gpsimd.dma_start(w_out[:, tb:tb + nj, :], sw)
                nc.sync.dma_start(i_out[:, tb:tb + nj, :], si)
                base += sgt
    return nc


_CACHE = {}


def _get_compiled(tpc):
    if tpc not in _CACHE:
        nc = build_nc(tpc)
        nc.compile()
        _CACHE[tpc] = nc
    return _CACHE[tpc]


def _prep_shared(weight, bias):
    f16 = np.float16
    f8 = ml_dtypes.float8_e4m3
    w = np.asarray(weight, np.float32)
    w_hi = w.astype(f16)
    w_lo22 = ((w - w_hi.astype(np.float32)) * (2.0 ** 22)).astype(f16)
    w8 = (w * 64.0).astype(f8)

    def wtile(a):  # [E, DIM] -> [128, KC, E]
        return np.ascontiguousarray(
            np.ascontiguousarray(a.T).reshape(KC, 128, E).transpose(1, 0, 2)
        )

    wa = np.empty((128, KC, 128), dtype=f16)
    wa[:, :, 0:64] = wtile(w_hi)
    wa[:, :, 64:128] = wtile(w_lo22)

    return {
        "wa": wa,
        "w8": wtile(w8),
        "ident": np.eye(64, dtype=np.float32),
        "identlo": (np.eye(64, dtype=np.float32) * LO_SCALE).astype(
            ml_dtypes.bfloat16
        ),
        "bias_b": np.ascontiguousarray(
            np.broadcast_to(np.asarray(bias, np.float32), (128, E))
        ),
        "iota64": np.ascontiguousarray(
            np.broadcast_to(np.arange(E, dtype=np.float32), (128, E))
        ),
    }


def prep_core_inputs(x, weight, bias, ncores=NCORES):
    f16 = np.float16
    f8 = ml_dtypes.float8_e4m3
    shared = _prep_shared(weight, bias)
    x = np.asarray(x, np.float32)
    tpc = x.shape[0] // ncores
    in_maps = []
    sgs = sg_layout(tpc)
    for c in range(ncores):
        xs = np.ascontiguousarray(x[c * tpc:(c + 1) * tpc].T)  # [DIM, tpc]
        xhi = xs.astype(f16)
        r = xs - xhi.astype(np.float32)
        lo8 = (r * 65536.0).astype(f8)
        # [DIM, tpc] -> per-group [128, KC, sgt]
        xh_pack = xhi.reshape(KC, 128, tpc).transpose(1, 0, 2)
        xl_pack = lo8.reshape(KC, 128, tpc).transpose(1, 0, 2)
        m = {**shared}
        base = 0
        for s, sgt in enumerate(sgs):
            m[f"xh{s}"] = np.ascontiguousarray(xh_pack[:, :, base:base + sgt])
            m[f"xl{s}"] = np.ascontiguousarray(xl_pack[:, :, base:base + sgt])
            base += sgt
        in_maps.append(m)
    return in_maps


def unpack_outputs(res_list, tpc):
    ws, idxs = [], []
    for r in res_list:
        wv = np.asarray(r["w_out"])  # [128, ntile, TOPK]
        iv = np.asarray(r["i_out"])
        ws.append(wv.transpose(1, 0, 2).reshape(tpc, TOPK))
        idxs.append(iv.transpose(1, 0, 2).reshape(tpc, TOPK))
    return (
        np.ascontiguousarray(np.concatenate(ws)).astype(np.float32),
        np.ascontiguousarray(np.concatenate(idxs)).astype(np.int32),
    )


def run(x, weight, bias, trace=False, **kwargs):
    x = np.asarray(x, np.float32)
    tpc = x.shape[0] // NCORES
    nc = _get_compiled(tpc)
    in_maps = prep_core_inputs(x, weight, bias)
    res = run_bass_kernel_spmd(nc, in_maps, list(range(NCORES)), trace=trace, **kwargs)
    w, i = unpack_outputs(res.results, tpc)
    return w, i, res


def kernel(x, weight, bias):
    w, i, _ = run(x, weight, bias, trace=False)
    return w, i
